# revision 3
# baseline (speedup 1.0000x reference)
"""LSTM autoencoder (4x LSTM H=512 + 2 Dense) on 8 TRN2 NeuronCores.

Strategy: batch-split layer pipeline.
  - Core pairs (0,1)=enc0, (2,3)=enc1, (4,5)=dec0, (6,7)=dec1; even cores
    take batch[0:32], odd cores batch[32:64].
  - h-chunks stream down the chain via AllGather on groups
    [[0,2,4,6],[1,3,5,7]] (slot select via per-core one-hot masks).
  - Each core: input projection (from received h / x-ext stream, bias via
    ones-row), LSTM recurrence (bf16 matmuls, f32 cell state), output
    projection (enc_dw / dec_dw per-core, zeros elsewhere).
  - dec0's input projection uses the fused weight enc_dw @ dec_k0 so
    `encoded` never has to be re-ingested.
  - All cores run one SPMD program; per-core behavior differs only via
    in_maps data (weights zeroed where unused).

Pipelining: iteration i runs recurrence on z[cur] (prepared in i-1), while
the projection for the next chunk (reading the AllGather output landed
during i-1..i) fills TensorE gaps; h is sent at end of i. Hop latency
between chain stages = 2 iterations. Depth-d core's chunk j finishes at
iteration j + 2d; host unshifts the output slots.
"""
import sys
import os

sys.path.insert(0, "/opt/trn_rl_repo")

import numpy as np
import ml_dtypes

import concourse.bass as bass
import concourse.bacc as bacc
import concourse.tile as tile
from concourse import mybir
from concourse.bass_utils import run_bass_kernel_spmd

BF = ml_dtypes.bfloat16

# Problem constants
B, T, D, H = 64, 1024, 32, 512
T = int(os.environ.get("AE_T", str(T)))  # dev override only
G = 4 * H          # 2048 gate dim
Bh = B // 2        # 32 per core
P = 128
KC = H // P        # 4 contraction chunks
MC = G // P        # 16 gate chunks
VC = H // P        # 4 out-proj chunks

# Tunables
C = int(os.environ.get("AE_C", "16"))          # timesteps per chunk
NCHUNK = T // C
NIT = NCHUNK + 6                               # 3 hops x 2 iterations
CW = C * Bh                                    # free size of one chunk (t,b)
NB = max(1, CW // 512)                         # 512-wide N blocks per chunk
NBW = CW // NB

_nc_cache = {}


def build_nc():
    key = (C, NIT)
    if key in _nc_cache:
        return _nc_cache[key]
    fp32, bf16, i32 = mybir.dt.float32, mybir.dt.bfloat16, mybir.dt.int32
    AF = mybir.ActivationFunctionType
    ALU = mybir.AluOpType

    nc = bacc.Bacc("TRN2", target_bir_lowering=False, debug=False, num_devices=8)

    wr_d = nc.declare_dram_parameter("wr", [H, G], bf16, isOutput=False)
    wk_d = nc.declare_dram_parameter("wk", [H, G], bf16, isOutput=False)
    wext_d = nc.declare_dram_parameter("wext", [64, G], bf16, isOutput=False)
    wout_d = nc.declare_dram_parameter("wout", [H, H], bf16, isOutput=False)
    msk_d = nc.declare_dram_parameter("msk", [P, 4], fp32, isOutput=False)
    xext_d = nc.declare_dram_parameter(
        "xext", [64, (NIT + 1) * CW], bf16, isOutput=False)
    out_d = nc.declare_dram_parameter("out", [H, NIT * CW], fp32, isOutput=True)

    with tile.TileContext(nc) as tc:
        with (
            tc.tile_pool(name="wpool", bufs=1) as wpool,
            tc.tile_pool(name="state", bufs=1) as state,
            tc.tile_pool(name="zpool", bufs=2) as zpool,
            tc.tile_pool(name="hpool", bufs=3) as hpool,
            tc.tile_pool(name="rpool", bufs=2) as rpool,
            tc.tile_pool(name="xpool", bufs=2) as xpool,
            tc.tile_pool(name="gpool", bufs=3) as gpool,
            tc.tile_pool(name="recps", bufs=2, space="PSUM") as recps,
            tc.tile_pool(name="pps", bufs=2, space="PSUM") as pps,
            tc.tile_pool(name="dram", bufs=2, space="DRAM") as dram,
        ):
            # ---- load weights (one-time) ----
            # wr_sb[:, 128*(4m+k)] = wr[128k:128k+128, 128m:+128]
            wr_sb = wpool.tile([P, KC * G], bf16, tag="wr")
            wrv = wr_sb[:].rearrange("p (m k om) -> p m k om", k=KC, om=P)
            wk_sb = wpool.tile([P, (KC + 1) * G], bf16, tag="wk")
            wkv = wk_sb[:].rearrange("p (m k om) -> p m k om", k=KC + 1, om=P)
            for k in range(KC):
                nc.sync.dma_start(
                    out=wrv[:, :, k, :],
                    in_=wr_d[k * P:(k + 1) * P, :].rearrange("p (m om) -> p m om", om=P))
                nc.sync.dma_start(
                    out=wkv[:, :, k, :],
                    in_=wk_d[k * P:(k + 1) * P, :].rearrange("p (m om) -> p m om", om=P))
            # ext weights as the 5th k chunk of wk (only 64 partitions used)
            nc.sync.dma_start(
                out=wkv[0:64, :, KC, :],
                in_=wext_d[:, :].rearrange("p (m om) -> p m om", om=P))
            wout_sb = wpool.tile([P, KC * H], bf16, tag="wout")
            woutv = wout_sb[:].rearrange("p (m k om) -> p m k om", k=KC, om=P)
            for k in range(KC):
                nc.sync.dma_start(
                    out=woutv[:, :, k, :],
                    in_=wout_d[k * P:(k + 1) * P, :].rearrange("p (m om) -> p m om", om=P))
            msk_sb = wpool.tile([P, 4], fp32, tag="msk")
            nc.sync.dma_start(out=msk_sb[:], in_=msk_d[:, :])

            # ---- state ----
            c_st = state.tile([P, KC * Bh], fp32, tag="c")
            nc.vector.memset(c_st[:], 0.0)

            # zero dram tile for AG bootstrap
            zero_sb = state.tile([P, CW * KC], bf16, tag="zero")
            nc.vector.memset(zero_sb[:], 0.0)
            ag_init = dram.tile([4 * P, CW * KC], bf16, tag="agout")
            agi = ag_init[:].rearrange("(s p) n -> s p n", s=4)
            for s in range(4):
                nc.sync.dma_start(out=agi[s, :, :], in_=zero_sb[:])

            h_init = hpool.tile([P, C * P], bf16, tag="h")
            nc.vector.memset(h_init[:], 0.0)

            def projection(it, zA, zB, prev_ag):
                """Prepare z[next] for iteration it+1 from AG output + ext."""
                recv4 = rpool.tile([P, 4 * CW * KC], bf16, tag="recv4")
                pav = prev_ag[:].rearrange("(s p) n -> s p n", s=4)
                for s in range(4):
                    nc.sync.dma_start(
                        out=recv4[:, s * CW * KC:(s + 1) * CW * KC], in_=pav[s, :, :])
                recv = rpool.tile([P, CW * KC], bf16, tag="recv")
                nc.vector.tensor_scalar_mul(
                    recv[:], recv4[:, 0:CW * KC], msk_sb[:, 0:1])
                for s in range(1, 4):
                    nc.vector.scalar_tensor_tensor(
                        recv[:], recv4[:, s * CW * KC:(s + 1) * CW * KC],
                        msk_sb[:, s:s + 1], recv[:], ALU.mult, ALU.add)
                # ext chunk for slot it+1
                xe = xpool.tile([64, CW], bf16, tag="xe")
                nc.sync.dma_start(
                    out=xe[:], in_=xext_d[:, (it + 1) * CW:(it + 2) * CW])

                rv = recv[:].rearrange("p (t k b) -> p t k b", k=KC, b=Bh)
                zAv = zA[:].rearrange("p (t m b) -> p t m b", m=MC // 2, b=Bh)
                zBv = zB[:].rearrange("p (t m b) -> p t m b", m=MC // 2, b=Bh)
                tpb = NBW // Bh  # timesteps per N block
                for m in range(MC):
                    for nb in range(NB):
                        ps = pps.tile([P, NBW], fp32, tag="pps")
                        t0, t1 = nb * tpb, (nb + 1) * tpb
                        for k in range(KC):
                            nc.tensor.matmul(
                                ps[:], wkv[:, m, k, :], rv[:, t0:t1, k, :],
                                start=(k == 0), stop=False)
                        nc.tensor.matmul(
                            ps[:], wkv[0:64, m, KC, :],
                            xe[:, nb * NBW:(nb + 1) * NBW],
                            start=False, stop=True)
                        dst = zAv if m < MC // 2 else zBv
                        nc.vector.tensor_copy(
                            dst[:, t0:t1, m % (MC // 2), :], ps[:])

            def recurrence(zA, zB, hbuf, hprev_buf):
                zAv = zA[:].rearrange("p (t n) -> p t n", n=(MC // 2) * Bh)
                zBv = zB[:].rearrange("p (t n) -> p t n", n=(MC // 2) * Bh)
                for t in range(C):
                    if t == 0:
                        hp = hprev_buf[:, (C - 1) * P:C * P]
                    else:
                        hp = hbuf[:, (t - 1) * P:t * P]
                    psA = recps.tile([P, (MC // 2) * Bh], fp32, tag="psA")
                    psB = recps.tile([P, (MC // 2) * Bh], fp32, tag="psB")
                    for half, ps in ((0, psA), (1, psB)):
                        for ml in range(MC // 2):
                            m = half * (MC // 2) + ml
                            for k in range(KC):
                                nc.tensor.matmul(
                                    ps[:, ml * Bh:(ml + 1) * Bh],
                                    wrv[:, m, k, :], hp[:, k * Bh:(k + 1) * Bh],
                                    start=(k == 0), stop=(k == KC - 1))
                    # z = psum + xz ; gates
                    nc.vector.tensor_add(psA[:], psA[:], zAv[:, t, :])
                    nc.vector.tensor_add(psB[:], psB[:], zBv[:, t, :])
                    HW = (MC // 4) * Bh  # 128: half of a z bank = i|f split
                    iF = gpool.tile([P, 2 * HW], fp32, tag="iF")
                    nc.scalar.activation(iF[:], psA[:], AF.Sigmoid)
                    gT = gpool.tile([P, HW], fp32, tag="gT")
                    nc.scalar.activation(gT[:], psB[:, 0:HW], AF.Tanh)
                    oS = gpool.tile([P, HW], fp32, tag="oS")
                    nc.scalar.activation(oS[:], psB[:, HW:2 * HW], AF.Sigmoid)
                    ig = gpool.tile([P, HW], fp32, tag="ig")
                    nc.vector.tensor_mul(ig[:], iF[:, 0:HW], gT[:])
                    nc.vector.tensor_mul(c_st[:], c_st[:], iF[:, HW:2 * HW])
                    nc.vector.tensor_add(c_st[:], c_st[:], ig[:])
                    tc_t = gpool.tile([P, HW], fp32, tag="tc")
                    nc.scalar.activation(tc_t[:], c_st[:], AF.Tanh)
                    nc.vector.tensor_mul(hbuf[:, t * P:(t + 1) * P], oS[:], tc_t[:])

            def outproj(it, hbuf):
                hv = hbuf[:].rearrange("p (t k b) -> p t k b", k=KC, b=Bh)
                tpb = NBW // Bh
                for m in range(VC):
                    for nb in range(NB):
                        ps = pps.tile([P, NBW], fp32, tag="pps")
                        t0, t1 = nb * tpb, (nb + 1) * tpb
                        for k in range(KC):
                            nc.tensor.matmul(
                                ps[:], woutv[:, m, k, :], hv[:, t0:t1, k, :],
                                start=(k == 0), stop=(k == KC - 1))
                        vst = gpool.tile([P, NBW], mybir.dt.float32, tag="vst")
                        nc.vector.tensor_copy(vst[:], ps[:])
                        nc.sync.dma_start(
                            out=out_d[m * P:(m + 1) * P,
                                      it * CW + nb * NBW: it * CW + (nb + 1) * NBW],
                            in_=vst[:])

            # ---- prologue: prepare z[cur] for iteration 0 ----
            zA_cur = zpool.tile([P, C * (MC // 2) * Bh], bf16, tag="zA")
            zB_cur = zpool.tile([P, C * (MC // 2) * Bh], bf16, tag="zB")
            projection(-1, zA_cur, zB_cur, ag_init)

            h_prev = h_init
            prev_ag = ag_init
            for it in range(NIT):
                hbuf = hpool.tile([P, C * P], bf16, tag="h")
                recurrence(zA_cur, zB_cur, hbuf, h_prev)
                if it < NIT - 1:
                    zA_nxt = zpool.tile([P, C * (MC // 2) * Bh], bf16, tag="zA")
                    zB_nxt = zpool.tile([P, C * (MC // 2) * Bh], bf16, tag="zB")
                    projection(it, zA_nxt, zB_nxt, prev_ag)
                    zA_cur, zB_cur = zA_nxt, zB_nxt
                outproj(it, hbuf)
                if it < NIT - 1:
                    ag_in = dram.tile([P, CW * KC], bf16, tag="agin")
                    nc.sync.dma_start(out=ag_in[:], in_=hbuf[:])
                    ag_out = dram.tile([4 * P, CW * KC], bf16, tag="agout")
                    nc.gpsimd.collective_compute(
                        "AllGather", mybir.AluOpType.bypass,
                        replica_groups=[[0, 2, 4, 6], [1, 3, 5, 7]],
                        ins=[ag_in[:].opt()], outs=[ag_out[:].opt()])
                    prev_ag = ag_out
                h_prev = hbuf

    nc.compile()
    _nc_cache[key] = nc
    return nc


def _bf(a):
    return np.ascontiguousarray(np.asarray(a, dtype=np.float32)).astype(BF)


def prep_in_maps(inputs):
    f32 = np.float32
    x = np.asarray(inputs["x"], f32)
    enc_k0, enc_r0, enc_b0 = (np.asarray(inputs[k], f32) for k in ("enc_k0", "enc_r0", "enc_b0"))
    enc_k1, enc_r1, enc_b1 = (np.asarray(inputs[k], f32) for k in ("enc_k1", "enc_r1", "enc_b1"))
    enc_dw, enc_db = np.asarray(inputs["enc_dw"], f32), np.asarray(inputs["enc_db"], f32)
    dec_k0, dec_r0, dec_b0 = (np.asarray(inputs[k], f32) for k in ("dec_k0", "dec_r0", "dec_b0"))
    dec_k1, dec_r1, dec_b1 = (np.asarray(inputs[k], f32) for k in ("dec_k1", "dec_r1", "dec_b1"))
    dec_dw, dec_db = np.asarray(inputs["dec_dw"], f32), np.asarray(inputs["dec_db"], f32)

    wk_dec0 = enc_dw @ dec_k0
    b_dec0 = enc_db @ dec_k0 + dec_b0

    zeros_kg = np.zeros((H, G), f32)
    zeros_hh = np.zeros((H, H), f32)
    wout_dec = np.zeros((H, H), f32)
    wout_dec[:, :D] = dec_dw

    # per layer: (wk, wr, xw, bias, wout)
    layer = [
        (zeros_kg, enc_r0, enc_k0, enc_b0, zeros_hh),
        (enc_k1, enc_r1, None, enc_b1, enc_dw),
        (wk_dec0, dec_r0, None, b_dec0, zeros_hh),
        (dec_k1, dec_r1, None, dec_b1, wout_dec),
    ]
    # masks: source slot = chain position - 1 (cores 0,1 have no source)
    slot_of = {0: None, 1: 0, 2: 1, 3: 2}

    in_maps = []
    for core in range(8):
        d = core // 2
        half = core % 2
        wk, wr, xw, bias, wout = layer[d]
        wext = np.zeros((64, G), f32)
        if xw is not None:
            wext[:D, :] = xw
        wext[D, :] = bias
        xext = np.zeros((64, (NIT + 1) * CW), f32)
        xext[D, :] = 1.0
        if d == 0:
            xh = x[half * Bh:(half + 1) * Bh]          # [Bh, T, D]
            xt = xh.transpose(2, 1, 0).reshape(D, NCHUNK, C * Bh)
            xext[:D, :NCHUNK * CW] = xt.reshape(D, NCHUNK * CW)
        msk = np.zeros((P, 4), f32)
        if slot_of[d] is not None:
            msk[:, slot_of[d]] = 1.0
        in_maps.append({
            "wr": _bf(wr), "wk": _bf(wk), "wext": _bf(wext),
            "wout": _bf(wout), "msk": msk, "xext": _bf(xext),
        })
    return in_maps


def run(inputs, trace=False, **kw):
    nc = build_nc()
    in_maps = prep_in_maps(inputs)
    res = run_bass_kernel_spmd(nc, in_maps, core_ids=list(range(8)), trace=trace, **kw)
    return res


def extract_outputs(results, inputs):
    enc_db = np.asarray(inputs["enc_db"], np.float32)
    dec_db = np.asarray(inputs["dec_db"], np.float32)

    def grab(core, depth, dim):
        o = results[core]["out"]  # [H, NIT*CW]
        lo = 2 * depth
        blk = o[:dim, lo * CW:(lo + NCHUNK) * CW]
        blk = blk.reshape(dim, NCHUNK, C, Bh)
        return blk.transpose(3, 1, 2, 0).reshape(Bh, T, dim)

    encoded = np.concatenate([grab(2, 1, H), grab(3, 1, H)], axis=0) + enc_db
    decoded = np.concatenate([grab(6, 3, D), grab(7, 3, D)], axis=0) + dec_db
    return encoded.astype(np.float32), decoded.astype(np.float32)


def kernel(**inputs):
    res = run(inputs, trace=False)
    return extract_outputs(res.results, inputs)


if __name__ == "__main__":
    nc = build_nc()
    print("build+compile OK")
